# revision 82
# baseline (speedup 1.0000x reference)
"""Trainium2 Bass kernel for CoAttention (bf16-optimized).

Math (per batch b), with host-folded weights:
    Am  = Wq.T @ Wk              (host, f64 -> bf16)
    c0  = Lq * (bq @ Wk)         (host, f64 -> bf16)
    w   = sum_q(sentence) @ Am + c0                 [D]
    s_k = comment[k] . w                            [Lk]
    p   = exp(s - max s);  l = sum p
    ctx = (p @ comment)                             [D]
    out = (ctx @ Wv.T + bv) / l                     [D]

Sharding: data-parallel over batch, 4 batches per core, weights replicated.
All bulk tensors ship as bf16 (halves the HBM/DMA floor; verified rel err
~5e-3 vs the 2e-2 gate). f32 accumulation everywhere (PSUM, DVE/ACT accums).

Engine plan per core (4 batches, comment shard 12.6 MB streamed once):
  - scores split per k-tile: ROUTE_A on DVE (fused mul+reduce ttr) and
    ROUTE_B as DVE mul (bf16 2x mode) -> ACT copy+accum reduce
  - softmax: DVE row-max, PE transpose/broadcast for the cross-partition
    max and sum, ACT exp with accum; each batch's chain is emitted a few
    ops into the next batch's score stream so the hops hide under it
  - w broadcast via PE outer product (ones x w_row), no DMA round trip
  - ctx accumulation + projections on PE; zero-add warm-up matmuls paced
    by score ops keep the PE p-state ramped into each ctx burst
  - all bulk DMA on the SP HWDGE ring in pinned order: am -> sentence ->
    comment -> wvt; engine queues pinned with zero-cost nosync deps
"""

import numpy as np

B, LQ, LK, D = 32, 512, 2048, 768
NCORES = 8
BPC = B // NCORES      # 4 batches per core
KT = LK // 128         # 16 k-tiles per batch
QT = LQ // 128         # 4 q-chunks per batch
DC = D // 128          # 6 d-chunks
KT_PER_SLAB = 4        # k-tiles per DMA slab
NSLAB = KT // KT_PER_SLAB  # 4 slabs per batch

# per-tile score route: DVE fused ttr / DVE mul + ACT reduce. (This walrus
# build cannot codegen any GPSIMD/Pool compute ops — "ISA wrong length".)
ROUTE_A = {0, 2, 4, 6, 9, 11, 13, 15}   # DVE scalar_tensor_tensor
ROUTE_B = {1, 3, 5, 7, 8, 10, 12, 14}   # DVE mult -> ACT copy+accum

_cache = {}


def _split_multi_waits(nc):
    """This walrus build allows only ONE sync-wait command per instruction.
    Tile emits several when an instruction depends on multiple procs. Hoist
    the extras onto same-engine NoOps inserted immediately before (the engine
    queue is FIFO, so the waits execute in order — semantically identical)."""
    import bass_rust
    from concourse import mybir

    n_split = 0
    for f in nc.m.functions:
        for bb in f.blocks:
            out = []
            for inst in bb.instructions:
                si = inst.sync_info
                waits = list(si.on_wait or []) if si else []
                if len(waits) > 1:
                    for i, w in enumerate(waits[:-1]):
                        nop = mybir.InstNoOp(name=f"{inst.name}-ws{i}")
                        nop.engine = inst.engine
                        nop.bass_nofuse = True
                        nop.sync_info = bass_rust.SyncInfo(
                            on_wait=[w], on_update=[]
                        )
                        out.append(nop)
                        n_split += 1
                    si.on_wait = waits[-1:]
                out.append(inst)
            bb.instructions[:] = out
    return n_split


def build_program(split_waits=True, reps=1):
    import contextlib

    import concourse.bass as bass
    import concourse.tile as tile
    from concourse import masks, mybir

    f32 = mybir.dt.float32
    bf16 = mybir.dt.bfloat16
    fp16 = mybir.dt.float16
    Alu = mybir.AluOpType
    Act = mybir.ActivationFunctionType
    Axis = mybir.AxisListType
    import concourse.bass as bass_mod

    nc = bass.Bass()
    sent = nc.declare_dram_parameter("sent", [BPC, LQ, D], bf16, isOutput=False)
    comm = nc.declare_dram_parameter("comm", [BPC, LK, D], bf16, isOutput=False)
    am = nc.declare_dram_parameter("am", [D, D], bf16, isOutput=False)
    wvt = nc.declare_dram_parameter("wvt", [D, D], bf16, isOutput=False)
    c0 = nc.declare_dram_parameter("c0", [D], bf16, isOutput=False)
    bv = nc.declare_dram_parameter("bv", [D], bf16, isOutput=False)
    out = nc.declare_dram_parameter("out", [BPC, D], f32, isOutput=True)

    sent_r = sent.rearrange("b (t p) d -> b p t d", p=128)  # [BPC,128,QT,D]
    comm_r = comm.rearrange("b (t p) d -> b p t d", p=128)  # [BPC,128,KT,D]
    am_r = am.rearrange("(c p) e -> p c e", p=128)          # [128,DC,D]
    wvt_r = wvt.rearrange("(c p) e -> p c e", p=128)

    with tile.TileContext(nc) as tc:
      rep_loop = tc.For_i(0, reps, 1) if reps > 1 else contextlib.nullcontext()
      with rep_loop:
        with (
            tc.tile_pool(name="consts", bufs=1) as consts,
            tc.tile_pool(name="rows", bufs=1) as rows,
            tc.tile_pool(name="smalls", bufs=2) as smalls,
            tc.tile_pool(name="wp", bufs=1) as wp,
            tc.tile_pool(name="sentp", bufs=1) as sentp,
            tc.tile_pool(name="slabp", bufs=BPC * NSLAB) as slabp,
            tc.tile_pool(name="wbp", bufs=4) as wbp,
            tc.tile_pool(name="prodp", bufs=4) as prodp,
            tc.tile_pool(name="dramp", bufs=1, space="DRAM") as dramp,
            tc.tile_pool(name="ps", bufs=1, space="PSUM") as ps,
        ):
            # ---------------- constants (no DMA) ----------------
            ident = consts.tile([128, 128], f32, name="ident")
            masks.make_identity(nc, ident[:])
            ident_bf = consts.tile([128, 128], bf16, name="ident_bf")
            nc.vector.tensor_copy(out=ident_bf[:], in_=ident[:])
            ones_f = consts.tile([128, 1], f32, name="ones_f")
            nc.vector.memset(ones_f[:], 1.0)
            ones_col = consts.tile([128, 1], bf16, name="ones_col")
            nc.vector.tensor_copy(out=ones_col[:], in_=ones_f[:])
            ones_row_f = consts.tile([1, 128], f32, name="ones_row_f")
            nc.vector.memset(ones_row_f[:], 1.0)
            ones_row_bf = consts.tile([1, 128], bf16, name="ones_row_bf")
            nc.vector.tensor_copy(out=ones_row_bf[:], in_=ones_row_f[:])
            ones4_f = consts.tile([1, BPC], f32, name="ones4_f")
            nc.vector.memset(ones4_f[:], 1.0)
            ones4 = consts.tile([1, BPC], bf16, name="ones4")
            nc.vector.tensor_copy(out=ones4[:], in_=ones4_f[:])
            dummy = consts.tile([1, 1], f32, name="dummy")
            nc.vector.memset(dummy[:], 0.0)
            nc.scalar.activation(dummy[:], dummy[:], Act.Exp)
            zcol2 = consts.tile([128, 2], bf16, name="zcol2")
            nc.vector.memset(zcol2[:].bitcast(f32), 0.0)

            # ---------------- DMA issue order (all bulk on SP) -----------
            # am -> sentence -> comment -> wvt; chained same-engine deps pin
            # the SP queue (and therefore the DMA-engine pool drain) order —
            # the tile scheduler otherwise reorders the issues and late
            # slabs head-of-line block earlier batches' dependents
            from concourse.bass import _add_dep_helper

            _dma_chain = [None]

            def chained_dma(out_ap, in_ap):
                inst = nc.sync.dma_start(out=out_ap, in_=in_ap)
                raw = getattr(inst, "ins", inst)
                if _dma_chain[0] is not None:
                    _add_dep_helper(raw, _dma_chain[0], sync=False,
                                    reason="pin SP DMA issue order")
                _dma_chain[0] = raw
                return inst

            am_sb = wp.tile([128, DC, D], bf16, name="am_sb")
            chained_dma(am_sb[:], am_r[:])

            sent_tiles = []
            for b in range(BPC):
                sent_sb = sentp.tile([128, QT, D], bf16, name=f"sent{b}")
                chained_dma(sent_sb[:], sent_r[b])
                sent_tiles.append(sent_sb)

            slabs = {}
            for b in range(BPC):
                for s in range(NSLAB):
                    t = slabp.tile([128, KT_PER_SLAB, D], bf16,
                                   name=f"slab{b}_{s}", tag="slab")
                    chained_dma(
                        t[:],
                        comm_r[b, :, s * KT_PER_SLAB:(s + 1) * KT_PER_SLAB, :],
                    )
                    slabs[(b, s)] = t

            wvt_sb = wp.tile([128, DC, D], bf16, name="wvt_sb")
            chained_dma(wvt_sb[:], wvt_r[:])

            # small loads on the ACT ring (bypass the bulk FIFO)
            c0_row = rows.tile([1, D], bf16, name="c0_row")
            nc.scalar.dma_start(out=c0_row[:], in_=c0[None, :])
            bv_row = rows.tile([1, D], bf16, name="bv_row")
            nc.scalar.dma_start(out=bv_row[:], in_=bv[None, :])

            # ---------------- phase 0: per-batch ssT -> w -> wb ----------
            # Entirely per-batch so wb[b] is ready ~2us after sentence b
            # lands. All matmuls keep the moving free size tiny (1) so the
            # cold-PE p-state ramp cannot hurt them.
            # seq(): pin per-engine execution order with zero-cost nosync
            # deps — the tile scheduler otherwise parks later batches'
            # slab-waiting ops ahead of earlier batches' chains in the
            # engine queues (head-of-line blocking)
            _eng_chain = {}

            def seq(bass_inst, eng):
                raw = getattr(bass_inst, "ins", bass_inst)
                prev = _eng_chain.get(eng)
                if prev is not None:
                    _add_dep_helper(raw, prev, sync=False,
                                    reason="pin engine order")
                _eng_chain[eng] = raw
                return bass_inst

            ssT = smalls.tile([128, DC, BPC], bf16, name="ssT", tag="ssT")
            wTb = smalls.tile([128, DC, BPC], bf16, name="wTb", tag="wTb")
            # ssT[:, c, b] = sum_q sent[b, q, c-chunk], sentence stationary
            # (moving free size 1 — immune to the cold-PE p-state ramp)
            for c in range(DC):
                pt = ps.tile([128, BPC], f32, name=f"ptc{c}", tag="C", bufs=2)
                for b in range(BPC):
                    for t in range(QT):
                        nc.tensor.matmul(
                            pt[:, b:b + 1],
                            sent_tiles[b][:, t, c * 128:(c + 1) * 128],
                            ones_col[:],
                            start=(t == 0), stop=(t == QT - 1))
                nc.vector.tensor_copy(out=ssT[:, c, :], in_=pt[:])
            # wT chunks: w = ssT.T @ Am + c0, am chunks stationary
            for m in range(DC):
                pw = ps.tile([128, BPC], f32, name=f"pw{m}", tag="C", bufs=2)
                for c in range(DC):
                    nc.tensor.matmul(
                        pw[:], am_sb[:, c, m * 128:(m + 1) * 128],
                        ssT[:, c, :], start=(c == 0), stop=False,
                    )
                nc.tensor.matmul(
                    pw[:], c0_row[0:1, m * 128:(m + 1) * 128],
                    ones4[0:1, :], start=False, stop=True,
                )
                nc.vector.tensor_copy(out=wTb[:, m, :], in_=pw[:])
            # transpose wT columns to partition-0 rows, then broadcast each
            # row down 128 partitions with a PE outer product (ones x w_row)
            # — no DMA round trip, so wb lands right after the w chain
            w_flat = rows.tile([1, BPC, D], bf16, name="w_flat")
            for b in range(BPC):
                pt2 = ps.tile([1, D], bf16, name=f"pt2_{b}",
                              tag="C2", bufs=2)
                for m in range(DC):
                    nc.tensor.transpose(pt2[0:1, m * 128:(m + 1) * 128],
                                        wTb[:, m, b:b + 1],
                                        ident_bf[:, 0:128])
                nc.scalar.copy(w_flat[0:1, b, :], pt2[:])
            wb_tiles = []
            for b in range(BPC):
                wb = wbp.tile([128, D], bf16, name=f"wb{b}", tag="wb")
                pba = ps.tile([128, 512], f32, name=f"pba{b}", tag="C",
                              bufs=2)
                nc.tensor.matmul(pba[:], ones_row_bf[:],
                                 w_flat[0:1, b, 0:512])
                pbb = ps.tile([128, 256], f32, name=f"pbb{b}", tag="C2",
                              bufs=2)
                nc.tensor.matmul(pbb[:], ones_row_bf[:],
                                 w_flat[0:1, b, 512:768])
                nc.scalar.copy(wb[:, 0:512], pba[:])
                nc.scalar.copy(wb[:, 512:768], pbb[:])
                wb_tiles.append(wb)

            # ---------------- main loop over batches ----------------
            # ctx PSUM is split into two 2-batch halves so the first half's
            # projection overlaps the second half's scores; masked p4 columns
            # land each batch in its own PSUM row (other row gets exact zeros)
            s_cols = smalls.tile([128, BPC, KT], f32, name="s_cols", tag="scols")
            li_row = rows.tile([1, BPC], f32, name="li_row")
            ca_h = [ps.tile([2, 512], f32, name=f"ca_h{h}", tag="A", bufs=2)
                    for h in range(2)]
            cb_h = [ps.tile([2, 256], f32, name=f"cb_h{h}", tag="Bk", bufs=2)
                    for h in range(2)]
            poa = ps.tile([BPC, 512], f32, name="poa", tag="A", bufs=2)
            pob = ps.tile([BPC, 256], f32, name="pob", tag="Bk", bufs=2)
            p4_tiles = []
            for b in range(BPC):
                p4 = smalls.tile([128, KT, 2], bf16, name=f"p4_{b}",
                                 tag=f"p4_{b}")
                nc.vector.memset(p4[:].bitcast(f32), 0.0)
                p4_tiles.append(p4)
            # ctxT halves: [128, DC, BPC] with the other half's columns zero
            ctxT_h = []
            for h in range(2):
                t_ = smalls.tile([128, DC, BPC], bf16, name=f"ctxT{h}",
                                 tag=f"ctxT{h}")
                nc.vector.memset(t_[:].bitcast(f32), 0.0)
                ctxT_h.append(t_)
            ctx_rows_h = [rows.tile([2, D], bf16, name=f"ctx_rows{h}")
                          for h in range(2)]

            def finalize_half(h):
                # copy ctx rows out of PSUM, transpose into the masked ctxT
                # half, and run each chunk's projection matmuls as soon as
                # that chunk's copy lands (per-chunk pipelining shortens the
                # tail after the last batch's ctx)
                cr = ctx_rows_h[h]
                nc.scalar.copy(cr[:, 0:512], ca_h[h][:])
                nc.scalar.copy(cr[:, 512:768], cb_h[h][:])
                for c in range(DC):
                    pt = ps.tile([128, 2], bf16, name=f"ptx{h}_{c}",
                                 tag="C", bufs=2)
                    nc.tensor.transpose(
                        pt[:], cr[:, c * 128:(c + 1) * 128],
                        ident_bf[0:2, 0:2],
                    )
                    nc.vector.tensor_copy(
                        out=ctxT_h[h][:, c, 2 * h:2 * h + 2], in_=pt[:])
                for c in range(DC):
                    nc.tensor.matmul(poa[:], ctxT_h[h][:, c, :],
                                     wvt_sb[:, c, 0:512],
                                     start=(h == 0 and c == 0), stop=False)
                    nc.tensor.matmul(pob[:], ctxT_h[h][:, c, :],
                                     wvt_sb[:, c, 512:768],
                                     start=(h == 0 and c == 0), stop=False)

            # dedicated per-engine garbage sinks for the unused elementwise
            # outputs: a single reused tile per engine adds only same-queue
            # WAW ordering (free) and avoids tile-pool rotation releases,
            # which the scheduler can place badly and stall score ops on

            # softmax + ctx emission for batch b, deferred into batch b+1's
            # score stream so the cross-engine max/sum hops hide under it
            rowmax_t = {}

            def emit_rowmax(b):
                rowmax = smalls.tile([128, 1], f32, name=f"rm{b}",
                                     tag="rowmax")
                seq(nc.vector.tensor_reduce(
                    out=rowmax[:], in_=s_cols[:, b, :], axis=Axis.X,
                    op=Alu.max
                ), "dve")
                prm = ps.tile([1, 128], f32, name=f"prm{b}", tag="C2", bufs=2)
                nc.tensor.transpose(prm[:], rowmax[:], ident[:])
                rm_row = smalls.tile([1, 128], f32, name=f"rmr{b}",
                                     tag="rmrow")
                seq(nc.scalar.copy(rm_row[:], prm[:]), "act")
                rowmax_t[b] = rm_row

            def emit_softmax_ctx(b):
                rm_row = rowmax_t[b]
                M_sb = smalls.tile([1, 1], f32, name=f"M{b}", tag="Msb")
                seq(nc.vector.tensor_reduce(
                    out=M_sb[:], in_=rm_row[:], axis=Axis.X, op=Alu.max
                ), "dve")
                pnm = ps.tile([128, 1], f32, name=f"pnm{b}", tag="C", bufs=2)
                nc.tensor.matmul(pnm[:], ones_row_f[:], M_sb[:])
                nm = smalls.tile([128, 1], f32, name=f"nm{b}", tag="nm")
                seq(nc.scalar.activation(nm[:], pnm[:], Act.Copy,
                                         scale=-1.0), "act")
                rowsum = smalls.tile([128, 1], f32, name=f"rs{b}",
                                     tag="rowsum")
                seq(nc.scalar.activation(
                    p4_tiles[b][:, :, b % 2], s_cols[:, b, :], Act.Exp,
                    bias=nm[:], scale=1.0, accum_out=rowsum[:],
                ), "act")
                pl = ps.tile([1, 1], f32, name=f"pl{b}", tag="C2", bufs=2)
                nc.tensor.matmul(pl[:], rowsum[:], ones_f[:])
                seq(nc.scalar.copy(li_row[0:1, b:b + 1], pl[:]), "act")
                # ctx accumulation on PE into this half's PSUM pair
                h = b // 2
                for t in range(KT):
                    ch = slabs[(b, t // KT_PER_SLAB)][:, t % KT_PER_SLAB, :]
                    pmat = p4_tiles[b][:, t, :]
                    nc.tensor.matmul(ca_h[h][:], pmat, ch[:, 0:512],
                                     start=(b % 2 == 0 and t == 0),
                                     stop=(b % 2 == 1 and t == KT - 1))
                    nc.tensor.matmul(cb_h[h][:], pmat, ch[:, 512:768],
                                     start=(b % 2 == 0 and t == 0),
                                     stop=(b % 2 == 1 and t == KT - 1))

            for b in range(BPC):
                wb = wb_tiles[b]
                # scores: DVE fused ttr / DVE mul + ACT reduce per k-tile
                for t in range(KT):
                    if b > 0 and t == 2:
                        emit_rowmax(b - 1)
                    if b > 0 and t == 5:
                        emit_softmax_ctx(b - 1)
                    ch = slabs[(b, t // KT_PER_SLAB)][:, t % KT_PER_SLAB, :]
                    s_col = s_cols[:, b, t:t + 1]
                    if t in ROUTE_A:
                        g = prodp.tile([128, D], fp16,
                                       name=f"gA{b}_{t}", tag="gA")
                        sc_inst = seq(nc.vector.scalar_tensor_tensor(
                            out=g[:], in0=ch, scalar=1.0, in1=wb[:],
                            op0=Alu.mult, op1=Alu.mult, accum_out=s_col,
                        ), "dve")
                        # PE warm-up: a zero-add into this half's open ctx
                        # accumulator (exactly +0; pre-start writes are
                        # discarded by the start reset). Paced by a dep on
                        # the score op so it runs during the score window,
                        # keeping the PE p-state ramped for the ctx burst.
                        wu = nc.tensor.matmul(
                            ca_h[b // 2][:], zcol2[:], ch[:, 0:512],
                            start=False, stop=False, skip_group_check=True)
                        _add_dep_helper(getattr(wu, "ins", wu),
                                        getattr(sc_inst, "ins", sc_inst),
                                        sync=True, reason="pe warmup pacing")
                    else:
                        prod = prodp.tile([128, D], fp16,
                                          name=f"pr{b}_{t}", tag="prod")
                        seq(nc.vector.tensor_tensor(
                            out=prod[:], in0=ch, in1=wb[:], op=Alu.mult),
                            "dve")
                        g2 = prodp.tile([128, D], fp16,
                                        name=f"gR{b}_{t}", tag="gR")
                        seq(nc.scalar.activation(
                            g2[:], prod[:], Act.Copy, accum_out=s_col),
                            "act")

                if b == 2:
                    # half 0 finalizes here: its deps (ctx b1) resolved while
                    # b2 streamed, and it doesn't gate b0-b2 score queues
                    finalize_half(0)

            emit_rowmax(BPC - 1)
            emit_softmax_ctx(BPC - 1)
            finalize_half(1)

            # ---------------- final projection tail ----------------
            nc.tensor.matmul(poa[:], ones4[0:1, :],
                             bv_row[0:1, 0:512],
                             start=False, stop=True)
            nc.tensor.matmul(pob[:], ones4[0:1, :],
                             bv_row[0:1, 512:768],
                             start=False, stop=True)

            pli = ps.tile([BPC, 1], f32, name="pli", tag="C", bufs=2)
            nc.tensor.transpose(pli[:], li_row[0:1, 0:BPC], ident[0:1, 0:1])
            li_col = smalls.tile([BPC, 1], f32, name="li_col", tag="li")
            nc.vector.tensor_copy(out=li_col[:], in_=pli[:])
            invl = smalls.tile([BPC, 1], f32, name="invl", tag="invl")
            nc.vector.reciprocal(invl[:], li_col[:])

            out_sb = rows.tile([BPC, D], f32, name="out_sb")
            nc.vector.tensor_scalar(
                out=out_sb[:, 0:512], in0=poa[:], scalar1=invl[:],
                scalar2=None, op0=Alu.mult,
            )
            nc.vector.tensor_scalar(
                out=out_sb[:, 512:768], in0=pob[:], scalar1=invl[:],
                scalar2=None, op0=Alu.mult,
            )
            nc.scalar.dma_start(out=out[:], in_=out_sb[:])

    if split_waits:
        _split_multi_waits(nc)
    return nc


def _get_program():
    if "nc" not in _cache:
        _cache["nc"] = build_program()
    return _cache["nc"]


def _make_in_maps(sentence_rep, comment_rep, Wq, bq, Wk, bk, Wv, bv):
    del bk  # softmax is shift-invariant: the bk term cancels exactly
    import ml_dtypes
    BF = ml_dtypes.bfloat16

    Wq64 = np.asarray(Wq, np.float64)
    Wk64 = np.asarray(Wk, np.float64)
    am = np.ascontiguousarray((Wq64.T @ Wk64).astype(BF))
    c0 = np.ascontiguousarray(
        (LQ * (np.asarray(bq, np.float64) @ Wk64)).astype(BF))
    wvt = np.ascontiguousarray(np.asarray(Wv, np.float32).T.astype(BF))
    bv_ = np.ascontiguousarray(np.asarray(bv, np.float32).astype(BF))
    sent = np.ascontiguousarray(np.asarray(sentence_rep, np.float32).astype(BF))
    comm = np.ascontiguousarray(np.asarray(comment_rep, np.float32).astype(BF))
    in_maps = []
    for c in range(NCORES):
        sl = slice(c * BPC, (c + 1) * BPC)
        in_maps.append({
            "sent": sent[sl], "comm": comm[sl],
            "am": am, "wvt": wvt, "c0": c0, "bv": bv_,
        })
    return in_maps


def run(inputs, trace=False, **kwargs):
    from concourse.bass_utils import run_bass_kernel_spmd

    nc = _get_program()
    in_maps = _make_in_maps(**inputs)
    res = run_bass_kernel_spmd(
        nc, in_maps, list(range(NCORES)), trace=trace, **kwargs
    )
    out = np.concatenate([res.results[c]["out"] for c in range(NCORES)], axis=0)
    return out.astype(np.float32), res


def kernel(**inputs) -> np.ndarray:
    out, _ = run(inputs)
    return out
